# revision 2
# baseline (speedup 1.0000x reference)
"""Trainium2 Bass kernel for EnhancedStrategySuperposition (MoE soft routing).

Math (per token b):
    logits = x @ W_att.T + b_att + adaptive_bias          [B, E]
    w      = softmax(logits + gumbel(u))                  [B, E]
    y[e]   = x @ W_strat[e].T + b_strat[e]                [B, E, A]
    out    = sum_e w[:, e] * y[e]                         [B, A]

Strategy:
  - Data-parallel: batch B=8192 sharded across 8 cores (1024 tokens each);
    gating + strategy weights replicated.
  - Host prep: transpose x and W_strat into [D, *] layouts (the PE contracts
    along the partition dim, so both matmul operands need D on partitions),
    cast to fp16 (PE runs fp16 at full bf16 rate; rel-err ~3e-4 vs fp32),
    fold b_att + adaptive_bias into the host-computed gumbel noise, so the
    device adds a single [B, E] tensor to the logits.
  - Device: per 128-token tile: gating matmul (fp16, K=1024, N=32) -> softmax
    (DVE reduce + ACT exp) -> per-expert weighted combine of the big matmul
    results via DVE scalar_tensor_tensor FMAs (per-partition scalar = w[:, e]).
    The b_strat term is handled exactly: transpose w on the PE and matmul
    wT @ b_strat (K=32) to seed the combine accumulator.
  - Big matmul: 8 d-chunks x 8 expert-groups (4 experts = 512 cols) per token
    tile, accumulating in PSUM, N=512 per matmul.
"""

import numpy as np

_B, _D, _E, _A = 8192, 1024, 32, 128
_NCORES = 8
_BL = _B // _NCORES  # tokens per core
_EPS = 1e-10

_cache = {}


def _build():
    """Build + compile the per-core Bass program (cached)."""
    if "nc" in _cache:
        return _cache["nc"]

    from contextlib import ExitStack

    from concourse import bacc, mybir, tile
    from concourse.bass import ts
    from concourse.masks import make_identity

    f16 = mybir.dt.float16
    f32 = mybir.dt.float32

    nc = bacc.Bacc("TRN2", debug=False, num_devices=_NCORES)

    xt_d = nc.dram_tensor("xt16", [_D, _BL], f16, kind="ExternalInput").ap()
    wt_d = nc.dram_tensor("wt16", [_D, _E * _A], f16, kind="ExternalInput").ap()
    wa_d = nc.dram_tensor("wa16", [_D, _E], f16, kind="ExternalInput").ap()
    g_d = nc.dram_tensor("g32", [_BL, _E], f32, kind="ExternalInput").ap()
    bs_d = nc.dram_tensor("bs32", [_E, _A], f32, kind="ExternalInput").ap()
    out_d = nc.dram_tensor("out", [_BL, _A], f32, kind="ExternalOutput").ap()

    KC = _D // 128  # contraction chunks
    JT = _BL // 128  # token tiles per core
    GG = _E // 4  # expert groups (4 experts x 128 cols = 512)

    with tile.TileContext(nc) as tc, ExitStack() as ctx:
        singles = ctx.enter_context(tc.tile_pool(name="singles", bufs=1))
        sb_small = ctx.enter_context(tc.tile_pool(name="small", bufs=3))
        sb_w = ctx.enter_context(tc.tile_pool(name="wts", bufs=2))
        sb_acc = ctx.enter_context(tc.tile_pool(name="accp", bufs=3))
        ps_logit = ctx.enter_context(tc.tile_pool(name="pslog", bufs=1, space="PSUM"))
        ps_wt = ctx.enter_context(tc.tile_pool(name="pswt", bufs=1, space="PSUM"))
        ps_b = ctx.enter_context(tc.tile_pool(name="psb", bufs=2, space="PSUM"))
        ps_big = ctx.enter_context(tc.tile_pool(name="psbig", bufs=4, space="PSUM"))

        # Resident inputs: x.T / W_strat.T / W_att.T chunked by 128 d-rows.
        x16 = []
        w16 = []
        wa16 = []
        for k in range(KC):
            xk = singles.tile([128, _BL], f16, tag=f"x{k}")
            nc.sync.dma_start(out=xk, in_=xt_d[ts(k, 128), :])
            x16.append(xk)
        for k in range(KC):
            wk = singles.tile([128, _E * _A], f16, tag=f"w{k}")
            nc.sync.dma_start(out=wk, in_=wt_d[ts(k, 128), :])
            w16.append(wk)
        for k in range(KC):
            ak = singles.tile([128, _E], f16, tag=f"wa{k}")
            nc.sync.dma_start(out=ak, in_=wa_d[ts(k, 128), :])
            wa16.append(ak)
        bs_sb = singles.tile([_E, _A], f32, tag="bs")
        nc.sync.dma_start(out=bs_sb, in_=bs_d[:, :])
        ident = singles.tile([128, 128], f32, tag="ident")
        make_identity(nc, ident)

        for j in range(JT):
            jsl = ts(j, 128)

            # ---- gating: logits = x @ W_att.T (+ bias folded into g) ----
            pl = ps_logit.tile([128, _E], f32, tag="logit")
            for k in range(KC):
                nc.tensor.matmul(
                    pl, x16[k][:, jsl], wa16[k], start=(k == 0), stop=(k == KC - 1)
                )
            gsb = sb_small.tile([128, _E], f32, tag="g")
            nc.sync.dma_start(out=gsb, in_=g_d[jsl, :])
            lg = sb_small.tile([128, _E], f32, tag="lg")
            nc.vector.tensor_add(lg, gsb, pl)

            # ---- softmax over E ----
            m = sb_small.tile([128, 1], f32, tag="m")
            nc.vector.reduce_max(m, lg, axis=mybir.AxisListType.X, negate=True)
            wsb = sb_w.tile([128, _E], f32, tag="w")
            s = sb_small.tile([128, 1], f32, tag="s")
            nc.scalar.activation(
                wsb,
                lg,
                mybir.ActivationFunctionType.Exp,
                bias=m,
                scale=1.0,
                accum_out=s,
            )
            rinv = sb_small.tile([128, 1], f32, tag="rinv")
            nc.vector.reciprocal(rinv, s)
            nc.vector.tensor_scalar_mul(wsb, wsb, rinv)

            # ---- bias term: pa0 = wT.T @ b_strat == sum_e w[:,e] b_strat[e,:] ----
            pwt = ps_wt.tile([32, 128], f32, tag="pwt")
            nc.tensor.transpose(pwt, wsb, ident)
            wt_sb = sb_small.tile([32, 128], f32, tag="wt")
            nc.vector.tensor_copy(wt_sb, pwt)
            pa0 = ps_b.tile([128, _A], f32, tag="pacc0")
            nc.tensor.matmul(pa0, wt_sb, bs_sb, start=True, stop=True)

            # ---- strategy matmuls + weighted combine ----
            acc = sb_acc.tile([128, _A], f32, tag="acc")
            nc.vector.tensor_copy(acc, pa0)  # seed with the b_strat term
            for gi in range(GG):
                ps = ps_big.tile([128, 512], f32, tag="big")
                for k in range(KC):
                    nc.tensor.matmul(
                        ps,
                        x16[k][:, jsl],
                        w16[k][:, ts(gi, 512)],
                        start=(k == 0),
                        stop=(k == KC - 1),
                    )
                for i in range(4):
                    e = gi * 4 + i
                    nc.vector.scalar_tensor_tensor(
                        out=acc,
                        in0=ps[:, ts(i, 128)],
                        scalar=wsb[:, e : e + 1],
                        in1=acc,
                        op0=mybir.AluOpType.mult,
                        op1=mybir.AluOpType.add,
                    )
            nc.sync.dma_start(out=out_d[jsl, :], in_=acc)

    nc.compile()
    _cache["nc"] = nc
    return nc


def _prep_in_maps(x, W_att, b_att, adaptive_bias, W_strat, b_strat, gumbel_u):
    x = np.asarray(x, dtype=np.float32)
    W_att = np.asarray(W_att, dtype=np.float32)
    b_att = np.asarray(b_att, dtype=np.float32)
    adaptive_bias = np.asarray(adaptive_bias, dtype=np.float32)
    W_strat = np.asarray(W_strat, dtype=np.float32)
    b_strat = np.asarray(b_strat, dtype=np.float32)
    gumbel_u = np.asarray(gumbel_u, dtype=np.float32)

    xT16 = np.ascontiguousarray(x.T).astype(np.float16)  # [D, B]
    wT16 = W_strat.transpose(2, 0, 1).reshape(_D, _E * _A).astype(np.float16)
    waT16 = np.ascontiguousarray(W_att.T).astype(np.float16)  # [D, E]
    bias_row = (b_att + adaptive_bias).astype(np.float32)
    g = -np.log(-np.log(gumbel_u + np.float32(_EPS)) + np.float32(_EPS))
    g = (g + bias_row[None, :]).astype(np.float32)
    bs32 = np.ascontiguousarray(b_strat, dtype=np.float32)

    in_maps = []
    for c in range(_NCORES):
        sl = slice(c * _BL, (c + 1) * _BL)
        in_maps.append(
            {
                "xt16": np.ascontiguousarray(xT16[:, sl]),
                "wt16": wT16,
                "wa16": waT16,
                "g32": np.ascontiguousarray(g[sl]),
                "bs32": bs32,
            }
        )
    return in_maps


def kernel(x, W_att, b_att, adaptive_bias, W_strat, b_strat, gumbel_u):
    assert x.shape == (_B, _D) and W_strat.shape == (_E, _A, _D)
    nc = _build()
    in_maps = _prep_in_maps(
        x, W_att, b_att, adaptive_bias, W_strat, b_strat, gumbel_u
    )
    from concourse.bass_utils import run_bass_kernel_spmd

    res = run_bass_kernel_spmd(nc, in_maps, list(range(_NCORES))).results
    out = np.concatenate([res[c]["out"] for c in range(_NCORES)], axis=0)
    return np.ascontiguousarray(out.astype(np.float32))
